# revision 8
# baseline (speedup 1.0000x reference)
"""Per-camera color calibration (grouped 1x1 conv == per-channel affine).

Full input: image [16,3,1024,1024] f32, camera_index [16] int,
weight/bias [34,3] f32.  out = image * weight[cam][:, :, None, None] + bias[...].

Strategy: data-parallel over batch across 8 cores (2 images/core).  The
34x3 tables are gathered host-side into per-(batch,channel) "plane"
coefficients (96 floats total); each core streams its 24 MiB shard
through SBUF in [128, FREE] tiles and applies a per-partition
tensor_scalar (mult, add) on the vector engine.  Memory-bound: 24 MiB in
+ 24 MiB out per core; roofline ~140 us at ~358 GB/s HBM per core.

Raw bass (no Tile): walrus codegen allows at most 1 sync-wait on the
TensorScalarPtr template, which Tile's auto-sem assignment exceeds.
Explicit standalone wait_ge instructions sidestep the limit entirely.

Pipeline per core (N_TILES=6 tiles of 4 MiB):
  SP  : load(t) -> in-slot t%BI   [waits ts(t-BI) done]
  DVE : ts(t): out-slot = in-slot * scale + bias
        [waits load(t) landed; store(t-BO) done reading out-slot]
  ACT : store(t) from out-slot t%BO  [waits ts(t) done]

Semaphores are per-slot so waits are exact-count (a single shared DMA
sem would be racy: the 16 SDMA engines increment independently, so a
cumulative count cannot prove one specific DMA completed).
"""

import numpy as np

import concourse.bass as bass
import concourse.mybir as mybir
from concourse.bass_utils import run_bass_kernel_spmd

N_CORES = 8
B = 16
C = 3
H = 1024
W = 1024
B_PER_CORE = B // N_CORES          # 2
PLANES = B_PER_CORE * C            # 6 planes of H*W per core
FREE = 8192                        # elems per partition per tile
TILE_ELEMS = 128 * FREE            # 1048576 == one plane per tile
N_TILES = (PLANES * H * W) // TILE_ELEMS  # 6
BI = 3                             # in-slot bufs
BO = 2                             # out-slot bufs

_nc_cache = None


def _build_nc(repeat=1):
    """Build the Bass module.  repeat>1 loops the whole pipeline `repeat`
    times over the same DRAM data — used only for benchmarking (amplifies
    device time over the per-call dispatch overhead); the shipped kernel
    uses repeat=1."""
    nc = bass.Bass(trn_type="TRN2", target_bir_lowering=False)
    f32 = mybir.dt.float32
    img_in = nc.dram_tensor("img_in", [N_TILES, 128, FREE], f32, kind="ExternalInput")
    coeff = nc.dram_tensor("coeff", [128, 2 * N_TILES], f32, kind="ExternalInput")
    img_out = nc.dram_tensor(
        "img_out", [N_TILES, 128, FREE], f32, kind="ExternalOutput"
    )

    with (
        nc.sbuf_tensor("ctile", [128, 2 * N_TILES], f32) as ctile,
        nc.sbuf_tensor("ibuf", [128, BI * FREE], f32) as ibuf,
        nc.sbuf_tensor("obuf", [128, BO * FREE], f32) as obuf,
        nc.semaphore("sem_c") as sem_c,
        nc.semaphore("sem_l0") as sem_l0,
        nc.semaphore("sem_l1") as sem_l1,
        nc.semaphore("sem_l2") as sem_l2,
        nc.semaphore("sem_s0") as sem_s0,
        nc.semaphore("sem_s1") as sem_s1,
        nc.semaphore("sem_v") as sem_v,
        nc.Block() as block,
    ):
        sem_l = [sem_l0, sem_l1, sem_l2]
        sem_s = [sem_s0, sem_s1]
        NG = N_TILES * repeat  # total pipeline steps

        def islot(g):
            b = g % BI
            return ibuf[:, b * FREE : (b + 1) * FREE]

        def oslot(g):
            b = g % BO
            return obuf[:, b * FREE : (b + 1) * FREE]

        @block.sync
        def _(sync):
            sync.dma_start(ctile[:, :], coeff[:, :]).then_inc(sem_c, 16)
            for g in range(NG):
                t = g % N_TILES
                if g >= BI:
                    # in-slot free once ts(g-BI) has read it
                    sync.wait_ge(sem_v, g - BI + 1)
                sync.dma_start(islot(g), img_in[t, :, :]).then_inc(
                    sem_l[g % BI], 16
                )

        @block.vector
        def _(vector):
            vector.wait_ge(sem_c, 16)
            for g in range(NG):
                t = g % N_TILES
                vector.wait_ge(sem_l[g % BI], 16 * (g // BI + 1))
                if g >= BO:
                    # out-slot free once store(g-BO) has read it
                    vector.wait_ge(sem_s[g % BO], 16 * (g // BO))
                vector.tensor_scalar(
                    oslot(g),
                    islot(g),
                    ctile[:, 2 * t : 2 * t + 1],
                    ctile[:, 2 * t + 1 : 2 * t + 2],
                    mybir.AluOpType.mult,
                    mybir.AluOpType.add,
                ).then_inc(sem_v, 1)

        @block.scalar
        def _(scalar):
            for g in range(NG):
                t = g % N_TILES
                scalar.wait_ge(sem_v, g + 1)
                scalar.dma_start(img_out[t, :, :], oslot(g)).then_inc(
                    sem_s[g % BO], 16
                )
            # make sure all stores have landed before the NEFF retires
            n0 = sum(1 for g in range(NG) if g % BO == 0)
            n1 = NG - n0
            scalar.wait_ge(sem_s[0], 16 * n0)
            scalar.wait_ge(sem_s[1], 16 * n1)

        # Block.__exit__ runs an all-engine barrier, after which every
        # wait above is satisfied; clear the sems so the NEFF is
        # re-executable.
        sems = [sem_c, sem_l0, sem_l1, sem_l2, sem_s0, sem_s1, sem_v]

    with nc.Block() as block2:

        @block2.sync
        def _(sync):
            for s in sems:
                sync.sem_clear(s)

    return nc


def _get_nc():
    global _nc_cache
    if _nc_cache is None:
        _nc_cache = _build_nc()
    return _nc_cache


def _run(image, camera_index, weight, bias, **spmd_kwargs):
    image = np.ascontiguousarray(np.asarray(image), dtype=np.float32)
    cam = np.asarray(camera_index).astype(np.int64)
    weight = np.asarray(weight, dtype=np.float32)
    bias = np.asarray(bias, dtype=np.float32)

    scale = weight[cam]   # [16, 3]
    shift = bias[cam]     # [16, 3]

    in_maps = []
    for c in range(N_CORES):
        lo = c * B_PER_CORE
        hi = lo + B_PER_CORE
        shard = image[lo:hi].reshape(N_TILES, 128, FREE)
        sc = scale[lo:hi].reshape(PLANES)
        sh = shift[lo:hi].reshape(PLANES)
        cf = np.empty((128, 2 * N_TILES), np.float32)
        cf[:, 0::2] = sc[None, :]
        cf[:, 1::2] = sh[None, :]
        in_maps.append({"img_in": shard, "coeff": cf})

    res = run_bass_kernel_spmd(
        _get_nc(), in_maps, core_ids=list(range(N_CORES)), **spmd_kwargs
    )
    out = np.concatenate(
        [r["img_out"].reshape(B_PER_CORE, C, H, W) for r in res.results], axis=0
    )
    return out, res


def kernel(image, camera_index, weight, bias):
    out, _ = _run(image, camera_index, weight, bias)
    return out


# revision 10
# speedup vs baseline: 1.0505x; 1.0505x over previous
"""Per-camera color calibration (grouped 1x1 conv == per-channel affine).

Full input: image [16,3,1024,1024] f32, camera_index [16] int,
weight/bias [34,3] f32.  out = image * weight[cam][:, :, None, None] + bias[...].

Strategy: data-parallel over batch across 8 cores (2 images/core).  The
34x3 tables are gathered host-side into per-(batch,channel) "plane"
coefficients (96 floats total); each core streams its 24 MiB shard
through SBUF in [128, FREE] tiles and applies a per-partition
tensor_scalar (mult, add) on the vector engine.  Memory-bound: 24 MiB in
+ 24 MiB out per core; roofline ~140 us at ~358 GB/s HBM per core.

Raw bass (no Tile): walrus codegen allows at most 1 sync-wait on the
TensorScalarPtr template, which Tile's auto-sem assignment exceeds.
Explicit standalone wait_ge instructions sidestep the limit entirely.

Pipeline per core (N_TILES tiles):
  SP  : load(t) -> in-slot t%BI   [waits ts(t-BI) done]
  DVE : ts(t): out-slot = in-slot * scale + bias
        [waits load(t) landed; store(t-BO) done reading out-slot]
  ACT : store(t) from out-slot t%BO  [waits ts(t) done]

Semaphores are per-slot so waits are exact-count (a single shared DMA
sem would be racy: the 16 SDMA engines increment independently, so a
cumulative count cannot prove one specific DMA completed).
"""

import numpy as np

import concourse.bass as bass
import concourse.mybir as mybir
from concourse.bass_utils import run_bass_kernel_spmd

N_CORES = 8
B = 16
C = 3
H = 1024
W = 1024
B_PER_CORE = B // N_CORES          # 2
PLANES = B_PER_CORE * C            # 6 planes of H*W per core
PLANE_ELEMS = H * W                # 1048576
FREE = 4096                        # elems per partition per tile
TILE_ELEMS = 128 * FREE            # 524288 == half a plane per tile
N_TILES = (PLANES * PLANE_ELEMS) // TILE_ELEMS  # 12
TPP = PLANE_ELEMS // TILE_ELEMS    # tiles per plane
BI = 6                             # in-slot bufs
BO = 5                             # out-slot bufs

_nc_cache = None


def _build_nc(repeat=1):
    """Build the Bass module.  repeat>1 loops the whole pipeline `repeat`
    times over the same DRAM data — used only for benchmarking (amplifies
    device time over the per-call dispatch overhead); the shipped kernel
    uses repeat=1."""
    nc = bass.Bass(trn_type="TRN2", target_bir_lowering=False)
    f32 = mybir.dt.float32
    img_in = nc.dram_tensor("img_in", [N_TILES, 128, FREE], f32, kind="ExternalInput")
    coeff = nc.dram_tensor("coeff", [128, 2 * N_TILES], f32, kind="ExternalInput")
    img_out = nc.dram_tensor(
        "img_out", [N_TILES, 128, FREE], f32, kind="ExternalOutput"
    )

    with (
        nc.sbuf_tensor("ctile", [128, 2 * N_TILES], f32) as ctile,
        nc.sbuf_tensor("ibuf", [128, BI * FREE], f32) as ibuf,
        nc.sbuf_tensor("obuf", [128, BO * FREE], f32) as obuf,
        nc.semaphore("sem_c") as sem_c,
        nc.semaphore("sem_v") as sem_v,
        _SemList(nc, "sem_l", BI) as sem_l,
        _SemList(nc, "sem_s", BO) as sem_s,
        nc.Block(no_gpsimd_drain=True) as block,
    ):
        NG = N_TILES * repeat  # total pipeline steps

        def islot(g):
            b = g % BI
            return ibuf[:, b * FREE : (b + 1) * FREE]

        def oslot(g):
            b = g % BO
            return obuf[:, b * FREE : (b + 1) * FREE]

        @block.sync
        def _(sync):
            sync.dma_start(ctile[:, :], coeff[:, :]).then_inc(sem_c, 16)
            for g in range(NG):
                t = g % N_TILES
                if g >= BI:
                    # in-slot free once ts(g-BI) has read it
                    sync.wait_ge(sem_v, g - BI + 1)
                sync.dma_start(islot(g), img_in[t, :, :]).then_inc(
                    sem_l[g % BI], 16
                )

        @block.vector
        def _(vector):
            vector.wait_ge(sem_c, 16)
            for g in range(NG):
                t = g % N_TILES
                vector.wait_ge(sem_l[g % BI], 16 * (g // BI + 1))
                if g >= BO:
                    # out-slot free once store(g-BO) has read it
                    vector.wait_ge(sem_s[g % BO], 16 * (g // BO))
                vector.tensor_scalar(
                    oslot(g),
                    islot(g),
                    ctile[:, 2 * t : 2 * t + 1],
                    ctile[:, 2 * t + 1 : 2 * t + 2],
                    mybir.AluOpType.mult,
                    mybir.AluOpType.add,
                ).then_inc(sem_v, 1)

        @block.scalar
        def _(scalar):
            for g in range(NG):
                t = g % N_TILES
                scalar.wait_ge(sem_v, g + 1)
                scalar.dma_start(img_out[t, :, :], oslot(g)).then_inc(
                    sem_s[g % BO], 16
                )
            # make sure all stores have landed before the NEFF retires
            for b in range(BO):
                nb = sum(1 for g in range(NG) if g % BO == b)
                scalar.wait_ge(sem_s[b], 16 * nb)

        sems = [sem_c, sem_v] + list(sem_l) + list(sem_s)

    with nc.Block(no_gpsimd_drain=True) as block2:

        @block2.sync
        def _(sync):
            for s in sems:
                sync.sem_clear(s)

    return nc


class _SemList:
    """Allocate n semaphores as one context manager."""

    def __init__(self, nc, name, n):
        self.nc = nc
        self.name = name
        self.n = n
        self._ctxs = []
        self._sems = []

    def __enter__(self):
        for i in range(self.n):
            ctx = self.nc.semaphore(f"{self.name}{i}")
            self._ctxs.append(ctx)
            self._sems.append(ctx.__enter__())
        return self._sems

    def __exit__(self, *a):
        for ctx in reversed(self._ctxs):
            ctx.__exit__(*a)
        return False


def _get_nc():
    global _nc_cache
    if _nc_cache is None:
        _nc_cache = _build_nc()
    return _nc_cache


def _make_in_maps(image, scale, shift):
    """Per-core input maps.  image [16,3,H,W] f32 contiguous; scale/shift
    [16,3] f32 (already gathered per sample)."""
    in_maps = []
    for c in range(N_CORES):
        lo = c * B_PER_CORE
        hi = lo + B_PER_CORE
        shard = image[lo:hi].reshape(N_TILES, 128, FREE)
        sc = np.repeat(scale[lo:hi].reshape(PLANES), TPP)  # [N_TILES]
        sh = np.repeat(shift[lo:hi].reshape(PLANES), TPP)
        cf = np.empty((128, 2 * N_TILES), np.float32)
        cf[:, 0::2] = sc[None, :]
        cf[:, 1::2] = sh[None, :]
        in_maps.append({"img_in": shard, "coeff": cf})
    return in_maps


def _run(image, camera_index, weight, bias, **spmd_kwargs):
    image = np.ascontiguousarray(np.asarray(image), dtype=np.float32)
    cam = np.asarray(camera_index).astype(np.int64)
    weight = np.asarray(weight, dtype=np.float32)
    bias = np.asarray(bias, dtype=np.float32)

    in_maps = _make_in_maps(image, weight[cam], bias[cam])

    res = run_bass_kernel_spmd(
        _get_nc(), in_maps, core_ids=list(range(N_CORES)), **spmd_kwargs
    )
    out = np.concatenate(
        [r["img_out"].reshape(B_PER_CORE, C, H, W) for r in res.results], axis=0
    )
    return out, res


def kernel(image, camera_index, weight, bias):
    out, _ = _run(image, camera_index, weight, bias)
    return out
